# revision 3
# baseline (speedup 1.0000x reference)
"""Quantized matmul (uint4 groupwise dequant) on 8 Trainium2 NeuronCores.

Computes out = a_f32 @ W where W[k, n] = (q[k, n] - zeros[k//128, n]) * scales[k//128, n].

Sharding: tensor-parallel along N (output features). Each of the 8 cores gets
N_LOCAL = 512 columns of q/scales/zeros and the full `a` (replicated). Each
core dequantizes its W slice to fp16 once into SBUF, then runs a dense
fp16 matmul with fp32 PSUM accumulation.

Device kernel layout choices (all host-side prep is pure layout/sharding):
 - `a` is fed pre-transposed and tiled as aT[m_out, k_in, k_out*128 + m_in]
   so each [128, 4096] SBUF tile is one contiguous 1 MiB DMA and slices
   [:, k*128:(k+1)*128] are matmul lhsT tiles (K on partitions).
 - scales/zeros rows are fed replicated across the 128 partitions of each
   k-group ([32, 128, 512]) so the dequant is plain elementwise DVE work.
"""

import numpy as np

M, K, N = 4096, 4096, 4096
G = 128          # quant group size
P = 128          # partitions
NCORES = 8
NL = N // NCORES          # 512 output columns per core
KT = K // P               # 32 k tiles (== quant groups)
MT = M // P               # 32 m tiles
MBLK = 8                  # m-tiles per PSUM block (8 PSUM banks)

_CACHE = {}


def _build_nc():
    import concourse.bacc as bacc
    import concourse.mybir as mybir
    import concourse.tile as tile

    f16 = mybir.dt.float16
    f32 = mybir.dt.float32
    i32 = mybir.dt.int32

    nc = bacc.Bacc("TRN2", target_bir_lowering=False, debug=False)

    aT = nc.dram_tensor("aT", [MT, P, K], f16, kind="ExternalInput").ap()
    q = nc.dram_tensor("q", [KT, P, NL], i32, kind="ExternalInput").ap()
    zb = nc.dram_tensor("zb", [KT, P, NL], f16, kind="ExternalInput").ap()
    sb = nc.dram_tensor("sb", [KT, P, NL], f16, kind="ExternalInput").ap()
    out = nc.dram_tensor("out", [MT, P, NL], f32, kind="ExternalOutput").ap()

    with tile.TileContext(nc) as tc:
        with (
            tc.tile_pool(name="w", bufs=KT) as wpool,
            tc.tile_pool(name="qraw", bufs=4) as qpool,
            tc.tile_pool(name="zs", bufs=8) as zspool,
            tc.tile_pool(name="deq", bufs=4) as dqpool,
            tc.tile_pool(name="at", bufs=12) as apool,
            tc.tile_pool(name="ot", bufs=4) as opool,
            tc.tile_pool(name="ps", bufs=MBLK, space="PSUM") as pspool,
        ):
            # Phase 1: dequantize W into resident SBUF fp16 tiles, one per k-group.
            w_tiles = []
            for k in range(KT):
                qt = qpool.tile([P, NL], i32)
                nc.sync.dma_start(qt[:], q[k])
                zt = zspool.tile([P, NL], f16, tag="z")
                nc.sync.dma_start(zt[:], zb[k])
                st = zspool.tile([P, NL], f16, tag="s")
                nc.sync.dma_start(st[:], sb[k])

                qf = dqpool.tile([P, NL], f16)
                nc.vector.tensor_copy(qf[:], qt[:])          # int32 -> fp16 cast
                d = dqpool.tile([P, NL], f16, tag="d")
                nc.vector.tensor_sub(out=d[:], in0=qf[:], in1=zt[:])
                wt = wpool.tile([P, NL], f16, tag="w")
                nc.vector.tensor_mul(out=wt[:], in0=d[:], in1=st[:])
                w_tiles.append(wt)

            # Phase 2: out[m, :] = sum_k aT[k, m].T @ W[k, :].
            # m-blocks of 8 PSUM banks, k outermost inside a block so the
            # PE only needs W_k tiles at the dequant pipeline's pace.
            from concourse.bass import ts

            for blk in range(MT // MBLK):
                ats = []
                for mi in range(MBLK):
                    at = apool.tile([P, K], f16)
                    nc.sync.dma_start(at[:], aT[blk * MBLK + mi])
                    ats.append(at)
                pss = [
                    pspool.tile([P, NL], f32, name=f"ps{blk}_{i}", tag="ps")
                    for i in range(MBLK)
                ]
                for k in range(KT):
                    for mi in range(MBLK):
                        nc.tensor.matmul(
                            pss[mi][:],
                            ats[mi][:, ts(k, P)],
                            w_tiles[k][:],
                            start=(k == 0),
                            stop=(k == KT - 1),
                        )
                for mi in range(MBLK):
                    ot = opool.tile([P, NL], f32)
                    nc.scalar.copy(ot[:], pss[mi][:])
                    nc.sync.dma_start(out[blk * MBLK + mi], ot[:])

    nc.compile()
    return nc


def _shard_inputs(a, q_weight, scales, zeros):
    """Host-side shard/layout. Pure slicing, transposition and replication."""
    # aT[m_out, k_in, k_out*128 + m_in] = a[m_out*128 + m_in, k_out*128 + k_in]
    aT = np.ascontiguousarray(
        a.reshape(MT, P, KT, P).transpose(0, 3, 2, 1)
    ).reshape(MT, P, K)

    in_maps = []
    for c in range(NCORES):
        sl = slice(c * NL, (c + 1) * NL)
        q_c = np.ascontiguousarray(q_weight[:, sl]).reshape(KT, P, NL)
        zb_c = np.ascontiguousarray(
            np.broadcast_to(zeros[:, None, sl], (KT, P, NL))
        )
        sb_c = np.ascontiguousarray(
            np.broadcast_to(scales[:, None, sl], (KT, P, NL))
        )
        in_maps.append({"aT": aT, "q": q_c, "zb": zb_c, "sb": sb_c})
    return in_maps


def _run(inputs, trace=False):
    from concourse import bass_utils

    if "nc" not in _CACHE:
        _CACHE["nc"] = _build_nc()
    nc = _CACHE["nc"]

    a = np.asarray(inputs["a"], dtype=np.float16)
    q_weight = np.asarray(inputs["q_weight"], dtype=np.int32)
    scales = np.asarray(inputs["scales"], dtype=np.float16)
    zeros = np.asarray(inputs["zeros"], dtype=np.float16)

    in_maps = _shard_inputs(a, q_weight, scales, zeros)
    res = bass_utils.run_bass_kernel_spmd(
        nc, in_maps, core_ids=list(range(NCORES)), trace=trace
    )

    out = np.empty((M, N), dtype=np.float32)
    for c in range(NCORES):
        out[:, c * NL : (c + 1) * NL] = res.results[c]["out"].reshape(M, NL)
    return out, res


def kernel(**inputs) -> np.ndarray:
    out, _ = _run(inputs, trace=False)
    return out


# revision 9
# speedup vs baseline: 1.0992x; 1.0992x over previous
"""Quantized matmul (uint4 groupwise dequant) on 8 Trainium2 NeuronCores.

Computes out = a_f32 @ W where W[k, n] = (q[k, n] - zeros[k//128, n]) * scales[k//128, n].

Sharding: tensor-parallel along N (output features). Each of the 8 cores gets
N_LOCAL = 512 columns of q/scales/zeros and the full `a` (replicated). Each
core dequantizes its W slice to fp16 once into SBUF, then runs a dense
fp16 matmul with fp32 PSUM accumulation.

Device kernel layout choices (all host-side prep is pure layout/sharding):
 - `a` is fed pre-transposed and tiled as aT[m_out, k_in, k_out*128 + m_in]
   so each [128, 4096] SBUF tile is one contiguous 1 MiB DMA and slices
   [:, k*128:(k+1)*128] are matmul lhsT tiles (K on partitions).
 - scales/zeros come in as [32, 512] slices; both are broadcast across the
   128 partitions on-device with a single stride-0 DRAM->SBUF DMA each
   (every partition's descriptor re-reads the same hot 32 KiB row).

Schedule: the first m-block's aT tiles are emitted before the dequant
pipeline so the PE can start as soon as W_0 is ready; the first block runs
k-outermost across 8 PSUM banks so the PE consumes W tiles at the dequant
pipeline's pace; later blocks run m-outer/k-inner with inline epilogues so
output DMAs spread out instead of bursting at the tail.
"""

import numpy as np

M, K, N = 4096, 4096, 4096
G = 128          # quant group size
P = 128          # partitions
NCORES = 8
NL = N // NCORES          # 512 output columns per core
KT = K // P               # 32 k tiles (== quant groups)
MT = M // P               # 32 m tiles
MBLK = 8                  # m-tiles in the k-outer first block (8 PSUM banks)

_CACHE = {}


def _build_nc():
    import concourse.bacc as bacc
    import concourse.mybir as mybir
    import concourse.tile as tile
    from concourse.bass import ts

    f16 = mybir.dt.float16
    f32 = mybir.dt.float32
    i32 = mybir.dt.int32

    nc = bacc.Bacc("TRN2", target_bir_lowering=False, debug=False)

    aT = nc.dram_tensor("aT", [MT, P, K], f16, kind="ExternalInput").ap()
    q = nc.dram_tensor("q", [KT, P, NL], i32, kind="ExternalInput").ap()
    zsm = nc.dram_tensor("zsm", [1, KT * NL], f16, kind="ExternalInput").ap()
    ssm = nc.dram_tensor("ssm", [1, KT * NL], f16, kind="ExternalInput").ap()
    out = nc.dram_tensor("out", [MT, P, NL], f32, kind="ExternalOutput").ap()

    with tile.TileContext(nc) as tc:
        with (
            tc.tile_pool(name="w", bufs=KT) as wpool,
            tc.tile_pool(name="zsb", bufs=1) as zsbpool,
            tc.tile_pool(name="qraw", bufs=4) as qpool,
            tc.tile_pool(name="deq", bufs=4) as dqpool,
            tc.tile_pool(name="at", bufs=10) as apool,
            tc.tile_pool(name="ot", bufs=4) as opool,
            tc.tile_pool(name="ps", bufs=MBLK, space="PSUM") as pspool,
        ):
            # Block-0 activations first: the PE's first work only needs these
            # plus W_0, so their DMAs go ahead of the bulk of the W pipeline.
            ats0 = []
            for mi in range(MBLK):
                at = apool.tile([P, K], f16, name=f"at0_{mi}", tag="at")
                nc.sync.dma_start(at[:], aT[mi])
                ats0.append(at)

            # One-shot partition broadcast of all groups' zeros/scales:
            # [1, KT*NL] flat DRAM view, stride-0 across 128 partitions.
            zb_all = zsbpool.tile([P, KT * NL], f16)
            nc.sync.dma_start(zb_all[:], zsm.partition_broadcast(P))
            sb_all = zsbpool.tile([P, KT * NL], f16)
            nc.sync.dma_start(sb_all[:], ssm.partition_broadcast(P))

            # Dequant pipeline: W[k] = (fp16(q[k]) - z[k]) * s[k], resident.
            w_tiles = []
            for k in range(KT):
                qt = qpool.tile([P, NL], i32)
                nc.sync.dma_start(qt[:], q[k])
                qf = dqpool.tile([P, NL], f16)
                nc.vector.tensor_copy(qf[:], qt[:])          # int32 -> fp16 cast
                d = dqpool.tile([P, NL], f16, tag="d")
                nc.vector.tensor_sub(out=d[:], in0=qf[:], in1=zb_all[:, ts(k, NL)])
                wt = wpool.tile([P, NL], f16, tag="w")
                nc.vector.tensor_mul(out=wt[:], in0=d[:], in1=sb_all[:, ts(k, NL)])
                w_tiles.append(wt)

            # Block 0: k outermost so the PE needs W tiles only at the
            # dequant pipeline's pace. 8 PSUM banks accumulate in parallel.
            pss = [
                pspool.tile([P, NL], f32, name=f"ps0_{i}", tag="ps")
                for i in range(MBLK)
            ]
            for k in range(KT):
                for mi in range(MBLK):
                    nc.tensor.matmul(
                        pss[mi][:],
                        ats0[mi][:, ts(k, P)],
                        w_tiles[k][:],
                        start=(k == 0),
                        stop=(k == KT - 1),
                    )
            for mi in range(MBLK):
                ot = opool.tile([P, NL], f32)
                nc.scalar.copy(ot[:], pss[mi][:])
                nc.sync.dma_start(out[mi], ot[:])

            # Remaining m-tiles: m-outer, k-inner, inline epilogue.
            for m in range(MBLK, MT):
                at = apool.tile([P, K], f16, name=f"at_{m}", tag="at")
                nc.sync.dma_start(at[:], aT[m])
                ps = pspool.tile([P, NL], f32, name=f"ps_{m}", tag="ps")
                for k in range(KT):
                    nc.tensor.matmul(
                        ps[:],
                        at[:, ts(k, P)],
                        w_tiles[k][:],
                        start=(k == 0),
                        stop=(k == KT - 1),
                    )
                ot = opool.tile([P, NL], f32)
                nc.scalar.copy(ot[:], ps[:])
                nc.sync.dma_start(out[m], ot[:])

    nc.compile()
    return nc


def _shard_inputs(a, q_weight, scales, zeros):
    """Host-side shard/layout. Pure slicing, transposition and replication."""
    # aT[m_out, k_in, k_out*128 + m_in] = a[m_out*128 + m_in, k_out*128 + k_in]
    aT = np.ascontiguousarray(
        a.reshape(MT, P, KT, P).transpose(0, 3, 2, 1)
    ).reshape(MT, P, K)

    in_maps = []
    for c in range(NCORES):
        sl = slice(c * NL, (c + 1) * NL)
        q_c = np.ascontiguousarray(q_weight[:, sl]).reshape(KT, P, NL)
        z_c = np.ascontiguousarray(zeros[:, sl]).reshape(1, KT * NL)
        s_c = np.ascontiguousarray(scales[:, sl]).reshape(1, KT * NL)
        in_maps.append({"aT": aT, "q": q_c, "zsm": z_c, "ssm": s_c})
    return in_maps


def _run(inputs, trace=False):
    from concourse import bass_utils

    if "nc" not in _CACHE:
        _CACHE["nc"] = _build_nc()
    nc = _CACHE["nc"]

    a = np.asarray(inputs["a"], dtype=np.float16)
    q_weight = np.asarray(inputs["q_weight"], dtype=np.int32)
    scales = np.asarray(inputs["scales"], dtype=np.float16)
    zeros = np.asarray(inputs["zeros"], dtype=np.float16)

    in_maps = _shard_inputs(a, q_weight, scales, zeros)
    res = bass_utils.run_bass_kernel_spmd(
        nc, in_maps, core_ids=list(range(NCORES)), trace=trace
    )

    out = np.empty((M, N), dtype=np.float32)
    for c in range(NCORES):
        out[:, c * NL : (c + 1) * NL] = res.results[c]["out"].reshape(M, NL)
    return out, res


def kernel(**inputs) -> np.ndarray:
    out, _ = _run(inputs, trace=False)
    return out


# revision 11
# speedup vs baseline: 1.1886x; 1.0814x over previous
"""Quantized matmul (uint4 groupwise dequant) on 8 Trainium2 NeuronCores.

Computes out = a_f32 @ W where W[k, n] = (q[k, n] - zeros[k//128, n]) * scales[k//128, n].

Sharding: tensor-parallel along N (output features). Each of the 8 cores gets
N_LOCAL = 512 columns of q/scales/zeros and the full `a` (replicated). Each
core dequantizes its W slice to fp16 once into SBUF, then runs a dense
fp16 matmul with fp32 PSUM accumulation.

Device kernel layout choices (all host-side prep is pure layout/sharding):
 - `a` is fed pre-transposed and tiled as aT[m_out, k_in, k_out*128 + m_in]
   so each [128, 4096] SBUF tile is one contiguous 1 MiB DMA and slices
   [:, k*128:(k+1)*128] are matmul lhsT tiles (K on partitions).
 - scales/zeros come in as [32, 512] slices; both are broadcast across the
   128 partitions on-device with a single stride-0 DRAM->SBUF DMA each
   (every partition's descriptor re-reads the same hot 32 KiB row).

Schedule: the first m-block's aT tiles are emitted before the dequant
pipeline so the PE can start as soon as W_0 is ready; the first block runs
k-outermost across 8 PSUM banks so the PE consumes W tiles at the dequant
pipeline's pace; later blocks run m-outer/k-inner with inline epilogues so
output DMAs spread out instead of bursting at the tail.
"""

import numpy as np

M, K, N = 4096, 4096, 4096
G = 128          # quant group size
P = 128          # partitions
NCORES = 8
NL = N // NCORES          # 512 output columns per core
KT = K // P               # 32 k tiles (== quant groups)
MT = M // P               # 32 m tiles
MBLK = 8                  # m-tiles in the k-outer first block (8 PSUM banks)

_CACHE = {}


def _build_nc():
    import concourse.bacc as bacc
    import concourse.mybir as mybir
    import concourse.tile as tile
    from concourse.bass import ts

    f16 = mybir.dt.float16
    f32 = mybir.dt.float32
    i32 = mybir.dt.int32

    nc = bacc.Bacc("TRN2", target_bir_lowering=False, debug=False)

    aT = nc.dram_tensor("aT", [MT, P, K], f16, kind="ExternalInput").ap()
    q = nc.dram_tensor("q", [KT, P, NL], i32, kind="ExternalInput").ap()
    zsm = nc.dram_tensor("zsm", [1, KT * NL], f16, kind="ExternalInput").ap()
    ssm = nc.dram_tensor("ssm", [1, KT * NL], f16, kind="ExternalInput").ap()
    out = nc.dram_tensor("out", [MT, P, NL], f32, kind="ExternalOutput").ap()

    with tile.TileContext(nc) as tc:
        GPC = 4                   # quant groups per broadcast chunk
        NCH = KT // GPC           # 8 broadcast chunks per tensor

        with (
            tc.tile_pool(name="w", bufs=KT) as wpool,
            tc.tile_pool(name="zsb", bufs=3) as zsbpool,
            tc.tile_pool(name="qraw", bufs=6) as qpool,
            tc.tile_pool(name="deq", bufs=4) as dqpool,
            tc.tile_pool(name="at", bufs=10) as apool,
            tc.tile_pool(name="ot", bufs=4) as opool,
            tc.tile_pool(name="ps", bufs=MBLK, space="PSUM") as pspool,
        ):
            # First m-tile's activations ahead of everything: the PE's very
            # first matmul needs them.
            ats0 = []
            at = apool.tile([P, K], f16, name="at0_0", tag="at")
            nc.sync.dma_start(at[:], aT[0])
            ats0.append(at)

            # W pipeline, chunked: each chunk broadcasts 4 groups of
            # zeros/scales across partitions (stride-0 DRAM read) and
            # dequantizes those 4 groups. Remaining block-0 aT loads are
            # interleaved so DMA bandwidth is shared the way the block-0
            # wavefront consumes it.
            w_tiles = []
            for j in range(NCH):
                zbc = zsbpool.tile([P, GPC * NL], f16, name=f"zbc{j}", tag="zb")
                nc.sync.dma_start(
                    zbc[:], zsm[:, j * GPC * NL : (j + 1) * GPC * NL].partition_broadcast(P)
                )
                sbc = zsbpool.tile([P, GPC * NL], f16, name=f"sbc{j}", tag="sb")
                nc.sync.dma_start(
                    sbc[:], ssm[:, j * GPC * NL : (j + 1) * GPC * NL].partition_broadcast(P)
                )
                for g in range(GPC):
                    k = j * GPC + g
                    qt = qpool.tile([P, NL], i32)
                    nc.sync.dma_start(qt[:], q[k])
                    qf = dqpool.tile([P, NL], f16)
                    nc.vector.tensor_copy(qf[:], qt[:])      # int32 -> fp16 cast
                    d = dqpool.tile([P, NL], f16, tag="d")
                    nc.vector.tensor_sub(out=d[:], in0=qf[:], in1=zbc[:, ts(g, NL)])
                    wt = wpool.tile([P, NL], f16, tag="w")
                    nc.vector.tensor_mul(out=wt[:], in0=d[:], in1=sbc[:, ts(g, NL)])
                    w_tiles.append(wt)
                if j < MBLK - 1:
                    at = apool.tile([P, K], f16, name=f"at0_{j + 1}", tag="at")
                    nc.sync.dma_start(at[:], aT[j + 1])
                    ats0.append(at)

            # Block 0: emit (mi, k) matmuls in availability order — aT tiles
            # land ~2.9 us apart, W groups ~1.3 us apart — so the PE stream
            # stalls as little as possible during the load ramp.
            pss = [
                pspool.tile([P, NL], f32, name=f"ps0_{i}", tag="ps")
                for i in range(MBLK)
            ]
            order = sorted(
                ((mi, k) for mi in range(MBLK) for k in range(KT)),
                key=lambda t: (max(2.9 * (t[0] + 1), 1.3 * (t[1] + 1)), t[0], t[1]),
            )
            for mi, k in order:
                nc.tensor.matmul(
                    pss[mi][:],
                    ats0[mi][:, ts(k, P)],
                    w_tiles[k][:],
                    start=(k == 0),
                    stop=(k == KT - 1),
                )
            for mi in range(MBLK):
                ot = opool.tile([P, NL], f32)
                nc.scalar.copy(ot[:], pss[mi][:])
                nc.sync.dma_start(out[mi], ot[:])

            # Remaining m-tiles: m-outer, k-inner, inline epilogue.
            for m in range(MBLK, MT):
                at = apool.tile([P, K], f16, name=f"at_{m}", tag="at")
                nc.sync.dma_start(at[:], aT[m])
                ps = pspool.tile([P, NL], f32, name=f"ps_{m}", tag="ps")
                for k in range(KT):
                    nc.tensor.matmul(
                        ps[:],
                        at[:, ts(k, P)],
                        w_tiles[k][:],
                        start=(k == 0),
                        stop=(k == KT - 1),
                    )
                ot = opool.tile([P, NL], f32)
                nc.scalar.copy(ot[:], ps[:])
                nc.sync.dma_start(out[m], ot[:])

    nc.compile()
    return nc


def _shard_inputs(a, q_weight, scales, zeros):
    """Host-side shard/layout. Pure slicing, transposition and replication."""
    # aT[m_out, k_in, k_out*128 + m_in] = a[m_out*128 + m_in, k_out*128 + k_in]
    aT = np.ascontiguousarray(
        a.reshape(MT, P, KT, P).transpose(0, 3, 2, 1)
    ).reshape(MT, P, K)

    in_maps = []
    for c in range(NCORES):
        sl = slice(c * NL, (c + 1) * NL)
        q_c = np.ascontiguousarray(q_weight[:, sl]).reshape(KT, P, NL)
        z_c = np.ascontiguousarray(zeros[:, sl]).reshape(1, KT * NL)
        s_c = np.ascontiguousarray(scales[:, sl]).reshape(1, KT * NL)
        in_maps.append({"aT": aT, "q": q_c, "zsm": z_c, "ssm": s_c})
    return in_maps


def _run(inputs, trace=False):
    from concourse import bass_utils

    if "nc" not in _CACHE:
        _CACHE["nc"] = _build_nc()
    nc = _CACHE["nc"]

    a = np.asarray(inputs["a"], dtype=np.float16)
    q_weight = np.asarray(inputs["q_weight"], dtype=np.int32)
    scales = np.asarray(inputs["scales"], dtype=np.float16)
    zeros = np.asarray(inputs["zeros"], dtype=np.float16)

    in_maps = _shard_inputs(a, q_weight, scales, zeros)
    res = bass_utils.run_bass_kernel_spmd(
        nc, in_maps, core_ids=list(range(NCORES)), trace=trace
    )

    out = np.empty((M, N), dtype=np.float32)
    for c in range(NCORES):
        out[:, c * NL : (c + 1) * NL] = res.results[c]["out"].reshape(M, NL)
    return out, res


def kernel(**inputs) -> np.ndarray:
    out, _ = _run(inputs, trace=False)
    return out
